# revision 16
# baseline (speedup 1.0000x reference)
"""Trainium2 Bass kernel for SAGAN-style self-attention (nn_Attention_36438502539877).

Reference computation (per batch b):
    x = inputs[b].reshape(4096, 256)
    f = x @ Wf + bf; g = x @ Wg + bg; h = x @ Wh + bh     # [4096, 32]
    beta = softmax(g @ f.T, axis=-1)                       # [4096, 4096]
    out = gamma * ((beta @ h) @ Wv + bv) + x

Sharding: 8 cores = 4 batches x 2 query-halves. Each core gets its batch's
full token set (keys/values) with rows rolled so its 2048 query rows sit at
rows 0..2047 (softmax + weighted key-sum are permutation invariant). gamma is
folded into Wv/bv on the host.

Per-core pipeline (everything transposed until the final output assembly):
  x^T via PE transposes -> packed projection fgh^T = [Wf|Wg|Wh]^T x^T.
  h^T re-transposed into h1[m, 33] tiles whose 33rd column is 1.0: the
  attention*value matmul then accumulates v^T AND the softmax denominator.
  s^T[m-tile, q] = f_tile^T g  (PSUM), exp on ACT (no max subtraction:
  |s| <= ~30 so fp32 exp is safe), acc[33, q] += h1^T exp(s^T).
  o^T_unnorm = [Wv; bv]^T acc  (bias via the denominator row - exact because
  out = (v_un @ Wv + denom*bv)/denom + x), PE-transpose to natural layout,
  multiply by 1/denom (per-partition scalar after a DMA partition-scatter of
  the denominator row), add residual, DMA out.

The attention-path matmuls run in bf16 (1 PE cycle/row; fp32r measures
2 cycles/row on hardware). The residual path (x, final add) stays fp32, so
with gamma == 0 the output equals x exactly.
"""

import os
import sys

for _p in ("/opt/trn_rl_repo", "/root/.axon_site/_ro/trn_rl_repo"):
    if os.path.isdir(_p) and _p not in sys.path:
        sys.path.insert(0, _p)

import numpy as np

import concourse.bass as bass
import concourse.mybir as mybir
import concourse.tile as tile
from concourse.masks import make_identity
from concourse.vector_clock import ScopedClock

F32 = mybir.dt.float32
BF16 = mybir.dt.bfloat16

N = 4096     # keys per core (full batch image)
Q = 2048     # queries per core
C = 256      # channels
CR = 32      # attention inner dim
NCHUNK = 512         # n-chunk width for x^T / projections
MTILES = N // 128    # 32 key tiles
QG = 1024            # query group width
NQG = Q // QG        # 2 query groups


class _TileContextSplitDrain(tile.TileContext):
    """TileContext with a post-pass splitting excess sem waits: this
    container's walrus rejects >1 sync wait on one instruction ("Too many
    sync wait commands"), so extra waits are hoisted onto standalone wait
    instructions on the same engine right before the instruction."""

    MAX_WAITS = 1

    def _split_excess_waits(self):
        import bass_rust

        nc = self.nc
        cur = nc.cur_bb.bb
        assert self.sems is not None
        id2h = {h.num: h for h in self.sems.allocated().values()}
        for f_ in nc.m.functions:
            for bb in f_.blocks:
                il = bb.instructions
                if not any(
                    inst.sync_info is not None
                    and inst.sync_info.on_wait
                    and len(inst.sync_info.on_wait) > self.MAX_WAITS
                    for inst in il
                ):
                    continue
                out = []
                for inst in il:
                    si = inst.sync_info
                    if si is not None and si.on_wait and len(si.on_wait) > self.MAX_WAITS:
                        waits = list(si.on_wait)
                        eng = nc.engines[inst.engine]
                        for w in waits[self.MAX_WAITS :]:
                            assert w.wait_mode == "sem-ge-imm", w
                            h = id2h.get(w.id) or bass_rust.SemaphoreHandle(
                                w.ant_name or f"S{w.id}", w.id
                            )
                            wi = eng.wait_ge(h, w.wait_value)
                            stolen = cur.instructions.pop()
                            assert stolen is wi.ins
                            out.append(stolen)
                        si.on_wait = waits[: self.MAX_WAITS]
                    out.append(inst)
                il[:] = out

    def _drain_and_barrier(self, tick_clock, wait_clock):
        nc = self.nc
        drain_inst = nc.sync.drain()
        wait_clock.add_sem_waits(
            drain_inst.ins, ScopedClock({None: tick_clock.global_clock})
        )
        self._split_excess_waits()
        nc.all_engine_barrier()
        popped = nc._tile_sem_poison_stack.pop()
        assert popped is self._sem_poison
        assert self.sems is not None
        nc.clear_and_free_semaphores(list(self.sems.allocated().values()))
        nc.all_engine_barrier()


def build_program(stage="full"):
    nc = bass.Bass("TRN2", target_bir_lowering=False, debug=False)

    x_d = nc.dram_tensor("x", [Q, C], F32, kind="ExternalInput").ap()
    xt_d = nc.dram_tensor("xt", [2, 128, N], BF16, kind="ExternalInput").ap()
    wfgh_d = nc.dram_tensor("wfgh", [C, 96], BF16, kind="ExternalInput").ap()
    bfgh_d = nc.dram_tensor("bfgh", [96, 1], F32, kind="ExternalInput").ap()
    wva_d = nc.dram_tensor("wva", [33, C], BF16, kind="ExternalInput").ap()
    out_d = nc.dram_tensor("out", [Q, C], F32, kind="ExternalOutput").ap()

    with _TileContextSplitDrain(nc) as tc:
        with (
            nc.allow_low_precision(reason="bf16 attention-path matmuls"),
            tc.tile_pool(name="singles", bufs=1) as singles,
            tc.tile_pool(name="expp", bufs=3) as expp,
            tc.tile_pool(name="small", bufs=2) as small,
            tc.tile_pool(name="outp", bufs=3) as outp,
            tc.tile_pool(name="ps_sT", bufs=2, space="PSUM") as ps_sT,
            tc.tile_pool(name="ps_acc", bufs=1, space="PSUM") as ps_acc,
            tc.tile_pool(name="ps_pro", bufs=2, space="PSUM") as ps_pro,
        ):
            # --- static SBUF tensors ---
            x_sb = singles.tile([128, Q // 128, C], F32, name="x_sb")
            xT0 = singles.tile([128, N], BF16, name="xT0")
            xT1 = singles.tile([128, N], BF16, name="xT1")
            f_sb = singles.tile([CR, N], BF16, name="f_sb")
            g_sb = singles.tile([CR, Q], BF16, name="g_sb")
            h_sb = singles.tile([CR, N], BF16, name="h_sb")
            h1_sb = singles.tile([128, MTILES, 33], BF16, name="h1_sb")
            wfgh_sb = singles.tile([128, 2, 96], BF16, name="wfgh_sb")
            bfgh_sb = singles.tile([96, 1], F32, name="bfgh_sb")
            wva_sb = singles.tile([33, C], BF16, name="wva_sb")
            ident = singles.tile([128, 128], F32, name="ident")
            identb = singles.tile([32, 32], BF16, name="identb")
            ones128 = singles.tile([128, CR], F32, name="ones128")

            # --- loads / init ---
            nc.sync.dma_start(
                out=wfgh_sb[:], in_=wfgh_d.rearrange("(j p) k -> p j k", p=128)
            )
            nc.sync.dma_start(out=bfgh_sb[:], in_=bfgh_d)
            nc.sync.dma_start(out=wva_sb[:], in_=wva_d)
            x_view = x_d.rearrange("(t p) c -> p t c", p=128)
            for gq in range(2):
                nc.sync.dma_start(
                    out=x_sb[:, 8 * gq : 8 * gq + 8, :],
                    in_=x_view[:, 8 * gq : 8 * gq + 8, :],
                )
            for j, xT in ((0, xT0), (1, xT1)):
                for gq in range(4):
                    nsl2 = slice(1024 * gq, 1024 * gq + 1024)
                    nc.sync.dma_start(out=xT[:, nsl2], in_=xt_d[j, :, nsl2])
            make_identity(nc, ident[:])
            make_identity(nc, identb[:])
            nc.vector.memset(ones128[:], 1.0)
            nc.vector.tensor_copy(h1_sb[:, :, 32], ones128[:])

            def produce_chunk(c):
                """Build x^T, f/g/h^T and h1 tiles for n-rows [512c, 512c+512)."""
                nsl = slice(NCHUNK * c, NCHUNK * (c + 1))
                pp = ps_pro.tile([96, 512], F32, tag="pro", name=f"pp{c}")
                nc.tensor.matmul(
                    pp[:], wfgh_sb[:, 0, :], xT0[:, nsl], start=True, stop=False
                )
                nc.tensor.matmul(
                    pp[:], wfgh_sb[:, 1, :], xT1[:, nsl], start=False, stop=True
                )
                nc.vector.tensor_scalar_add(f_sb[:, nsl], pp[0:32, :], bfgh_sb[0:32, :])
                if c < 4:  # g only needed for this core's 2048 queries
                    nc.vector.tensor_scalar_add(
                        g_sb[:, nsl], pp[32:64, :], bfgh_sb[32:64, :]
                    )
                nc.vector.tensor_scalar_add(h_sb[:, nsl], pp[64:96, :], bfgh_sb[64:96, :])
                # h1 tiles 4c..4c+3: h^T [32, 128] -> [128, 32], ones col stays
                ph = ps_pro.tile([128, 128], BF16, tag="pro", name=f"ph{c}")
                for k in range(4):
                    i = 4 * c + k
                    nc.tensor.matmul(
                        ph[:, 32 * k : 32 * k + 32],
                        h_sb[:, 128 * i : 128 * i + 128],
                        identb[:],
                        is_transpose=True,
                    )
                nc.vector.tensor_copy(h1_sb[:, 4 * c : 4 * c + 4, 0:32], ph[:])

            # chunk production schedule interleaved into qg0's m-loop: chunks
            # {0,1} before m-tile 0 (qg0 needs g chunks 0,1), then one chunk
            # every 4 m-tiles, staying ahead of f/h consumption.
            psched = {0: [0, 1], 4: [2], 8: [3], 12: [4], 16: [5], 20: [6], 24: [7]}

            exp_fn = mybir.ActivationFunctionType.Exp

            if stage == "pro":
                for c in range(8):
                    produce_chunk(c)
                dbg = outp.tile([128, C], F32, tag="outt", name="dbg")
                nc.vector.tensor_copy(dbg[0:32, :], f_sb[:, 0:256])
                nc.vector.tensor_copy(dbg[32:64, :], g_sb[:, 0:256])
                nc.vector.tensor_copy(dbg[64:96, :], h_sb[:, 0:256])
                nc.vector.tensor_copy(dbg[96:128, :], xT0[0:32, 0:256])
                nc.sync.dma_start(out=out_d[0:128, :], in_=dbg[:])
                return nc

            def mm1_exp(qg, i):
                sT = ps_sT.tile([128, QG], F32, tag="sT", name=f"sT{qg}_{i}")
                for h2 in range(2):
                    qsl = slice(QG * qg + 512 * h2, QG * qg + 512 * h2 + 512)
                    nc.tensor.matmul(
                        sT[:, 512 * h2 : 512 * h2 + 512],
                        f_sb[:, 128 * i : 128 * i + 128],
                        g_sb[:, qsl],
                    )
                expt = expp.tile([128, QG], BF16, tag="expt", name=f"expt{qg}_{i}")
                nc.scalar.activation(expt[:], sT[:], exp_fn)
                return expt

            def mm2(qg, i, expt, acc):
                for h2 in range(2):
                    esl = slice(512 * h2, 512 * h2 + 512)
                    nc.tensor.matmul(
                        acc[:, esl],
                        h1_sb[:, i, :],
                        expt[:, esl],
                        start=(i == 0),
                        stop=(i == MTILES - 1),
                    )

            def epilogue(qg, acc):
                # v^T_unnorm + denominator -> SBUF (bf16)
                vv = small.tile([33, QG], BF16, tag="vv", name=f"vv{qg}")
                nc.vector.tensor_copy(vv[:], acc[:])
                # denominator row scattered to natural per-query layout
                dn = small.tile([128, QG // 128], BF16, tag="dn", name=f"dn{qg}")
                for t in range(QG // 128):
                    nc.sync.dma_start(
                        out=dn[:, t : t + 1], in_=vv[32:33, 128 * t : 128 * t + 128]
                    )
                rcp = small.tile([128, QG // 128], F32, tag="rcp", name=f"rcp{qg}")
                nc.vector.reciprocal(rcp[:], dn[:])
                # o^T_unnorm = [Wv; bv]^T @ [v_un; denom]
                oT_sbs = []
                for half in range(2):
                    oT_ps = ps_sT.tile([128, QG], F32, tag="sT", name=f"oTps{qg}_{half}")
                    for h2 in range(2):
                        esl = slice(512 * h2, 512 * h2 + 512)
                        nc.tensor.matmul(
                            oT_ps[:, esl],
                            wva_sb[:, 128 * half : 128 * half + 128],
                            vv[:, esl],
                        )
                    oT_sb = small.tile([128, QG], F32, tag="oTsb", name=f"oTsb{qg}_{half}")
                    nc.vector.tensor_copy(oT_sb[:], oT_ps[:])
                    oT_sbs.append(oT_sb)
                # back to natural layout, normalize, residual, store
                for t in range(QG // 128):
                    po = ps_pro.tile([128, 256], F32, tag="pro", name=f"po{qg}_{t}")
                    for half in range(2):
                        nc.tensor.matmul(
                            po[:, 128 * half : 128 * half + 128],
                            oT_sbs[half][:, 128 * t : 128 * t + 128],
                            ident[:],
                            is_transpose=True,
                        )
                    om = outp.tile([128, C], F32, tag="om", name=f"om{qg}_{t}")
                    nc.vector.tensor_scalar_mul(om[:], po[:], rcp[:, t : t + 1])
                    outt = outp.tile([128, C], F32, tag="outt", name=f"outt{qg}_{t}")
                    nc.vector.tensor_add(outt[:], om[:], x_sb[:, (QG // 128) * qg + t, :])
                    row0 = QG * qg + 128 * t
                    nc.sync.dma_start(out=out_d[row0 : row0 + 128, :], in_=outt[:])

            # ---- query group 0 (with interleaved prologue) ----
            acc0 = ps_acc.tile([33, QG], F32, tag="acc", name="acc0")
            for i in range(MTILES):
                for c in psched.get(i, ()):
                    produce_chunk(c)
                expt = mm1_exp(0, i)
                if stage == "mm1":
                    if i == MTILES - 1:
                        dbg = outp.tile([128, C], F32, tag="outt", name="dbgm1")
                        nc.vector.tensor_copy(dbg[:], expt[:, 0:256])
                        nc.sync.dma_start(out=out_d[0:128, :], in_=dbg[:])
                    continue
                mm2(0, i, expt, acc0)
            if stage == "mm1":
                return nc

            # ---- query group 1, with qg0's epilogue interleaved into its
            # first m-tiles so ACT never starves while PE runs the epilogue ----
            acc1 = ps_acc.tile([33, QG], F32, tag="acc", name="acc1")
            PRE = 3
            pre = [mm1_exp(1, i) for i in range(PRE)]
            epilogue(0, acc0)
            for i in range(PRE):
                mm2(1, i, pre[i], acc1)
            for i in range(PRE, MTILES):
                expt = mm1_exp(1, i)
                mm2(1, i, expt, acc1)
            epilogue(1, acc1)

    return nc


_NC = None


def _get_nc():
    global _NC
    if _NC is None:
        _NC = build_program()
    return _NC


def _host_prep(inputs, Wf, bf, Wg, bg, Wh, bh, Wv, bv, gamma):
    import ml_dtypes

    x = np.asarray(inputs, np.float32).reshape(4, N, C)
    wfgh = np.concatenate(
        [np.asarray(Wf, np.float32), np.asarray(Wg, np.float32), np.asarray(Wh, np.float32)],
        axis=1,
    ).astype(ml_dtypes.bfloat16)  # [256, 96]
    bfgh = np.concatenate(
        [np.asarray(bf, np.float32), np.asarray(bg, np.float32), np.asarray(bh, np.float32)]
    ).reshape(96, 1)
    gma = np.float32(np.asarray(gamma).reshape(-1)[0])
    wva = np.concatenate(
        [np.asarray(Wv, np.float32) * gma, (np.asarray(bv, np.float32) * gma)[None, :]],
        axis=0,
    ).astype(ml_dtypes.bfloat16)  # [33, 256] = [gamma*Wv; gamma*bv]
    in_maps = []
    for core in range(8):
        b, qh = divmod(core, 2)
        xb = x[b]
        if qh:
            xb = np.roll(xb, -qh * Q, axis=0)
        xt = np.ascontiguousarray(
            xb.T.reshape(2, 128, N).astype(ml_dtypes.bfloat16)
        )  # [2, 128, N]: xt[j, p, n] = xb[n, 128j+p]
        in_maps.append(
            {
                "x": np.ascontiguousarray(xb[:Q]),
                "xt": xt,
                "wfgh": wfgh,
                "bfgh": bfgh,
                "wva": wva,
            }
        )
    return in_maps


def _gather(results, inputs_shape, dtype):
    out = np.empty((4, N, C), np.float32)
    for core in range(8):
        b, qh = divmod(core, 2)
        out[b, qh * Q : (qh + 1) * Q, :] = results[core]["out"]
    return out.reshape(inputs_shape).astype(dtype, copy=False)


def kernel(**inputs):
    from concourse.bass_utils import run_bass_kernel_spmd

    in_maps = _host_prep(**inputs)
    nc = _get_nc()
    res = run_bass_kernel_spmd(nc, in_maps, list(range(8)))
    x_in = np.asarray(inputs["inputs"])
    return _gather(res.results, x_in.shape, x_in.dtype)


def kernel_profiled(**inputs):
    """Like kernel() but with NTFF tracing; returns (out, BassKernelResults)."""
    import types

    if "antenv.axon_hooks" not in sys.modules:
        mod = types.ModuleType("antenv.axon_hooks")
        mod._h = None
        mod.set_axon_ntff_profile_hook = lambda h: setattr(mod, "_h", h)
        mod.get_axon_ntff_profile_hook = lambda: mod._h
        sys.modules["antenv.axon_hooks"] = mod
        try:
            from trn_agent_boot.trn_boot import _ntff_profile_via_ctypes

            mod._h = _ntff_profile_via_ctypes("/opt/axon/libaxon_pjrt.so")
        except Exception as e:  # profiling unavailable; run untraced
            print("NTFF hook unavailable:", e)
    from concourse.bass_utils import run_bass_kernel_spmd

    in_maps = _host_prep(**inputs)
    nc = _get_nc()
    res = run_bass_kernel_spmd(nc, in_maps, list(range(8)), trace=True)
    x_in = np.asarray(inputs["inputs"])
    return _gather(res.results, x_in.shape, x_in.dtype), res
